# revision 12
# baseline (speedup 1.0000x reference)
"""Trainium2 Bass kernel for nn_BucketedGoWatti (sparse windowed attention pooling).

Math (B=4, L=4096, T=32, DH=1024, DG=256, DP=256, WIN=1024, STRIDE=256, W=13):
  q  = G @ Wq_core;  logits[b,t,l] = (q @ Wk_core^T) . H[b,l]  (window-independent)
  alpha = softmax of logits restricted to window; Zw[b,t,w,:] = alpha @ Hw
  Since windows are 4 consecutive 256-chunks, Zw[w] = (P[w]+P[w+1]+P[w+2]+P[w+3])/den
  with P[c] = sum_{l in chunk c} exp(logit[t,l]) * H[l,:]  and den from per-chunk
  exp-sums. Device computes P[c] + csum[c] only; window composition, the tiny
  cross-window softmax (qw2 = (G@Wq_win)@Wk_win^T) and the final combine run on host.

Sharding: core c -> batch b=c//2, l-half c%2 (disjoint 2048 rows of H, zero halo).
Each core streams H once in each orientation (bf16): HT (d-major) for logits,
Hn (l-major) for P. Host pre-packs both layouts so every DMA moves 1MB with
8KB-contiguous per-partition descriptors.
"""
import numpy as np
import ml_dtypes
from contextlib import ExitStack

import concourse.bacc as bacc
import concourse.tile as tile
import concourse.mybir as mybir
import concourse.masks as masks
from concourse.bass_utils import run_bass_kernel_spmd

F32 = mybir.dt.float32
BF16 = mybir.dt.bfloat16
FP8 = mybir.dt.float8e4
ActFn = mybir.ActivationFunctionType

B, L, T = 4, 4096, 32
DH, DG, DP = 1024, 256, 256
WIN, STRIDE = 1024, 256
W = (L - WIN) // STRIDE + 1      # 13
SPAN = 2048                      # per-core l-span (disjoint)
NSLAB = 4                        # 512-l logits slabs
NDT = 8                          # d-tiles of 128
NCH = 8                          # 256-l chunks per core
NLT = 16                         # 128-l tiles per core
NGRP = 2                         # P output groups (4 chunks each, packed to 128 parts)

_CACHE = {}


def _build(with_mask: bool):
    nc = bacc.Bacc("TRN2", debug=False, target_bir_lowering=False)

    HT_d = nc.dram_tensor("HTl", [128, NSLAB * NDT * 512], BF16, kind="ExternalInput")
    Hn_d = nc.dram_tensor("Hnl", [128, 4 * 4 * DH], BF16, kind="ExternalInput")
    QKT_d = nc.dram_tensor("QKT", [128, NDT * T], BF16, kind="ExternalInput")
    if with_mask:
        mb_d = nc.dram_tensor("maskbias", [1, SPAN], BF16, kind="ExternalInput")
    P_d = nc.dram_tensor("P_out", [NGRP * 128, DH], BF16, kind="ExternalOutput")
    cs_d = nc.dram_tensor("csum_out", [T, NCH], F32, kind="ExternalOutput")

    with tile.TileContext(nc) as tc, ExitStack() as ctx:
        const = ctx.enter_context(tc.tile_pool(name="const", bufs=1))
        hpool = ctx.enter_context(tc.tile_pool(name="hpool", bufs=1))
        spool = ctx.enter_context(tc.tile_pool(name="spool", bufs=1))
        lg = ctx.enter_context(tc.tile_pool(name="lg", bufs=2, space="PSUM"))
        tp = ctx.enter_context(tc.tile_pool(name="tp", bufs=2, space="PSUM"))
        zp = ctx.enter_context(tc.tile_pool(name="zp", bufs=4, space="PSUM"))

        identb = const.tile([128, 128], BF16, tag="identb")
        masks.make_identity(nc, identb[:])
        qkt = const.tile([128, NDT * T], BF16, tag="qkt")
        nc.scalar.dma_start(qkt[:], QKT_d.ap())
        if with_mask:
            onesr = const.tile([1, T], BF16, tag="onesr")
            mbias = const.tile([1, SPAN], BF16, tag="mbias")
            nc.gpsimd.memset(onesr[:], 1.0)
            nc.scalar.dma_start(mbias[:], mb_d.ap())

        ht = [hpool.tile([128, NDT * 512], BF16, tag=f"ht{s}", name=f"ht{s}")
              for s in range(NSLAB)]
        hn = [hpool.tile([128, 2 * DH], BF16, tag=f"hn{g}", name=f"hn{g}")
              for g in range(NCH)]
        for s in range(NSLAB):
            nc.sync.dma_start(ht[s][:], HT_d.ap()[:, s * 4096:(s + 1) * 4096])
        for g in range(NCH):
            nc.sync.dma_start(hn[g][:], Hn_d.ap()[:, g * 2048:(g + 1) * 2048])

        csum = spool.tile([T, NCH], F32, tag="csum")
        expL = [spool.tile([T, 512], BF16, tag=f"expL{s}", name=f"expL{s}")
                for s in range(NSLAB)]
        expLT = [spool.tile([128, T], BF16, tag=f"eT{j}", name=f"eT{j}")
                 for j in range(NLT)]
        pstage = [spool.tile([128, DH], BF16, tag=f"pst{g}", name=f"pst{g}")
                  for g in range(NGRP)]

        # PE warmup: dummy matmuls bridge the pre-data idle gap so the HAM
        # clock gate opens (2.4 GHz) before the real matmuls begin.
        wsrc = spool.tile([128, 512], BF16, tag="wsrc")
        nc.gpsimd.memset(wsrc[:], 0.0)
        wu = lg.tile([128, 512], F32, tag="lg", name="wu")
        for k in range(8):
            nc.tensor.matmul(wu[:], identb[:], wsrc[:], start=True, stop=True)

        # phase 1: all logits matmuls (gated only by ht DMA arrivals)
        lgt = []
        for s in range(NSLAB):
            ps = lg.tile([T, 512], F32, tag="lg", name=f"lg{s}")
            lgt.append(ps)
            for i in range(NDT):
                nc.tensor.matmul(ps[:], qkt[:, i * T:(i + 1) * T],
                                 ht[s][:, i * 512:(i + 1) * 512],
                                 start=(i == 0), stop=(i == NDT - 1 and not with_mask))
            if with_mask:
                nc.tensor.matmul(ps[:], onesr[:], mbias[:, s * 512:(s + 1) * 512],
                                 start=False, stop=True)

        # phase 2: exps (scalar engine, runs concurrently with later logits)
        for s in range(NSLAB):
            for u in range(2):
                c = 2 * s + u
                nc.scalar.activation(expL[s][:, u * 256:(u + 1) * 256],
                                     lgt[s][:, u * 256:(u + 1) * 256],
                                     ActFn.Exp, accum_out=csum[:, c:c + 1])
        nc.scalar.dma_start(cs_d.ap(), csum[:])

        # phase 3: transposes (single-pass bf16) + bf16 copies
        for j in range(NLT):
            s, jj = j // 4, j % 4
            tps = tp.tile([128, T], BF16, tag="tp")
            nc.tensor.transpose(tps[:], expL[s][:, jj * 128:(jj + 1) * 128],
                                identb[:T, :T])
            nc.vector.tensor_copy(expLT[j][:], tps[:])

        # phase 4: P chunks (each gated by its own 512KB hn DMA),
        # packed 4 chunks -> 128 partitions of one PSUM pair
        zpt = {}
        for c in range(NCH):
            grp, q = c // 4, c % 4
            if q == 0:
                zpt[(grp, 0)] = zp.tile([128, 512], F32, tag="zp",
                                        name=f"zp{grp}a")
                zpt[(grp, 1)] = zp.tile([128, 512], F32, tag="zp",
                                        name=f"zp{grp}b")
            for lt in range(2):
                j = 2 * c + lt
                for h in range(2):
                    nc.tensor.matmul(zpt[(grp, h)][q * 32:(q + 1) * 32, :],
                                     expLT[j][:],
                                     hn[c][:, lt * DH + h * 512:lt * DH + (h + 1) * 512],
                                     start=(lt == 0), stop=(lt == 1),
                                     tile_position=(0, q * 32))
            if q == 3:
                nc.vector.tensor_copy(pstage[grp][:, 0:512], zpt[(grp, 0)][:])
                nc.scalar.activation(pstage[grp][:, 512:1024], zpt[(grp, 1)][:],
                                     ActFn.Copy)
                nc.scalar.dma_start(P_d.ap()[grp * 128:(grp + 1) * 128, :],
                                    pstage[grp][:])

    nc.compile()
    return nc


def kernel(H, G, Wq_core, Wk_core, Wq_win, Wk_win, attn_mask):
    H = np.asarray(H, dtype=np.float32)
    G = np.asarray(G, dtype=np.float32)
    Wq_core = np.asarray(Wq_core, dtype=np.float32)
    Wk_core = np.asarray(Wk_core, dtype=np.float32)
    Wq_win = np.asarray(Wq_win, dtype=np.float32)
    Wk_win = np.asarray(Wk_win, dtype=np.float32)
    mask = np.asarray(attn_mask).astype(bool)

    with_mask = not bool(mask.all())
    key = ("k", with_mask)
    if key not in _CACHE:
        _CACHE[key] = _build(with_mask)
    nc = _CACHE[key]

    # host precompute of the tiny query-side projections (f64 for accuracy)
    G64 = G.astype(np.float64)
    QK = (G64 @ Wq_core.astype(np.float64)) @ Wk_core.astype(np.float64).T
    QK *= DP ** -0.5                                    # [B, T, DH]
    qw2 = (G64 @ Wq_win.astype(np.float64)) @ Wk_win.astype(np.float64).T
    qw2 *= DH ** -0.5                                   # [B, T, DH]

    Hb = H.astype(ml_dtypes.bfloat16)
    in_maps = []
    for c in range(8):
        b, half = c // 2, c % 2
        hs = Hb[b, half * SPAN:(half + 1) * SPAN]       # [2048, 1024] bf16
        Hn_l = np.ascontiguousarray(
            hs.reshape(4, 4, 128, DH).transpose(2, 0, 1, 3).reshape(128, 16384))
        HT_l = np.ascontiguousarray(
            hs.reshape(4, 512, 8, 128).transpose(3, 0, 2, 1).reshape(128, 16384))
        QKT_l = np.ascontiguousarray(
            QK[b].T.reshape(8, 128, T).transpose(1, 0, 2).reshape(128, 8 * T)
        ).astype(ml_dtypes.bfloat16)
        im = {"HTl": HT_l, "Hnl": Hn_l, "QKT": QKT_l}
        if with_mask:
            im["maskbias"] = np.where(
                mask[b, half * SPAN:(half + 1) * SPAN], 0.0, -1e9
            ).astype(ml_dtypes.bfloat16)[None, :]
        in_maps.append(im)

    import os
    prof_dir = os.environ.get("BGW_PROFILE_DIR")
    if prof_dir:
        res = run_bass_kernel_spmd(nc, in_maps, core_ids=list(range(8)),
                                   trace=True, tmpdir=prof_dir)
    else:
        res = run_bass_kernel_spmd(nc, in_maps, core_ids=list(range(8)))
    kernel._last_result = res

    # host combine: window sums of chunk partials, then tiny W=13 softmax
    Z = np.empty((B, T, DH), dtype=np.float32)
    for b in range(B):
        Pc, css = [], []
        for half in range(2):
            r = res.results[2 * b + half]
            Pc.append(np.asarray(r["P_out"]).astype(np.float32)
                      .reshape(NGRP * 4, T, DH))
            css.append(np.asarray(r["csum_out"]).astype(np.float32))
        P = np.concatenate(Pc, axis=0)                  # [16, T, DH]
        cs = np.concatenate(css, axis=1)                # [T, 16]
        S = P[0:13] + P[1:14] + P[2:15] + P[3:16]       # [13, T, DH]
        den = cs[:, 0:13] + cs[:, 1:14] + cs[:, 2:15] + cs[:, 3:16]   # [T, 13]
        Zw = S / den.T[:, :, None]                      # [13, T, DH]
        wlog = np.einsum('wtd,td->tw', Zw, qw2[b])
        wlog -= wlog.max(axis=1, keepdims=True)
        e = np.exp(wlog)
        wsm = e / e.sum(axis=1, keepdims=True)          # [T, 13]
        Z[b] = np.einsum('tw,wtd->td', wsm, Zw)
    return Z
